# revision 7
# baseline (speedup 1.0000x reference)
"""Coref mention-ranking head on 8 TRN2 NeuronCores (Bass/Tile).

Math (reference): for mention i and antecedent slot c in [0, 50):
    J = max(0, i-50) + c, valid iff c < min(i, 50)
    combined = [cur_i, ant_J, cur_i*ant_J, dist_emb[clip(i-J,0,9)]]
    score = relu(combined @ W1 + b1) @ W2 + b2
    out[i, 0] = 0; out[i, c+1] = score (masked to 0 if invalid)

Decomposition (s = i - J in [1, 50] is the "shift"):
    z(i,s) = (cur_i*ant_{i-s}) @ W1c + ZA[i] + ZB[i-s] + zdf[min(s,9)] + b1
    score(i,s) = sign(W2) . relu(|W2| * z(i,s))   (|W2| folded into W1/b1)
with ZA = emb @ W1a, ZB = emb @ W1b computed once per mention.
Device computes the dense grid score[s, i]; host scatters it into slots.

Per-output-column scale S_m (a power of two) is folded into all W1
pieces so the first KF8 k-tiles of the bilinear term can run in
fp8e4m3 with DoubleRow perf mode (2 k-tiles per pass); the rest run
bf16. 1/S_m is folded exactly into the second-layer weights.

The additive terms (S*(ZA+ZB+zdf+b1)) are PRELOADED into PSUM with a
single scalar_tensor_tensor pass per tile, and the matmuls accumulate
on top (start=False), eliminating one elementwise pass vs computing
post-matmul. Elementwise work is split across DVE / Pool / ACT.

Sharding: mention axis split across 8 cores (256 mentions each);
weights replicated. Each core receives a 306-column transposed
embedding window [n0-50, n0+256) (zero-padded for core 0).
"""

from contextlib import ExitStack

import numpy as np
import ml_dtypes

import concourse.bass as bass
import concourse.bacc as bacc
import concourse.tile as tile
from concourse import mybir
from concourse.bass_utils import run_bass_kernel_spmd

F32 = mybir.dt.float32
BF16 = mybir.dt.bfloat16
F8 = mybir.dt.float8e4
RELU = mybir.ActivationFunctionType.Relu
DR = mybir.MatmulPerfMode.DoubleRow
ADD = mybir.AluOpType.add

N = 2048      # mentions
H = 1024      # hidden
A = 50        # max antecedents
FEAT = 20
NCORES = 8
NLOC = N // NCORES          # 256 mentions per core
W = NLOC + A                # 306-column embedding window per core
KT = H // 128               # 8 h_in tiles
MT = H // 128               # 8 h_out tiles
NBLK = A // 2               # 25 blocks of 2 shifts x 256 mentions
KF8 = 4                     # k-tiles (of KT) done in fp8 DoubleRow (even)
KB16 = KT - KF8             # k-tiles done in bf16
GRP = 4                     # blocks sharing one stationary-load sweep
NB = 10                     # distance buckets


def _build_nc():
    nc = bacc.Bacc("TRN2", target_bir_lowering=False, debug=False)

    embq_d = nc.dram_tensor("embq", [128, KT * 2 * W], BF16,
                            kind="ExternalInput")
    w1cf8_d = nc.dram_tensor("w1cf8", [128, (KF8 // 2) * 2 * H], F8,
                             kind="ExternalInput")
    w1cb16_d = nc.dram_tensor("w1cb16", [128, KB16 * H], BF16,
                              kind="ExternalInput")
    w1a_d = nc.dram_tensor("w1a", [H, H], BF16, kind="ExternalInput")
    w1b_d = nc.dram_tensor("w1b", [H, H], BF16, kind="ExternalInput")
    w1db1_d = nc.dram_tensor("w1db1", [FEAT + 1, H], F32, kind="ExternalInput")
    distT1_d = nc.dram_tensor("distT1", [FEAT + 1, NB], F32,
                              kind="ExternalInput")
    sgnp_d = nc.dram_tensor("sgnp", [128, MT], BF16, kind="ExternalInput")
    scores_d = nc.dram_tensor("scores", [NBLK, 512], F32,
                              kind="ExternalOutput")

    with tile.TileContext(nc) as tc, ExitStack() as ctx:
        const = ctx.enter_context(tc.tile_pool(name="const", bufs=1))
        wab = ctx.enter_context(tc.tile_pool(name="wab", bufs=3))
        xpool = ctx.enter_context(tc.tile_pool(name="x", bufs=8))
        htpool = ctx.enter_context(tc.tile_pool(name="ht", bufs=6))

        embq = const.tile([128, KT, 2, W], BF16)
        nc.sync.dma_start(embq[:], embq_d.rearrange(
            "p (k j w) -> p k j w", k=KT, j=2))
        w1cf8 = const.tile([128, KF8 // 2, 2, H], F8)
        nc.sync.dma_start(w1cf8[:], w1cf8_d.rearrange(
            "p (t s m) -> p t s m", t=KF8 // 2, s=2))
        w1cb16 = const.tile([128, KB16, H], BF16)
        nc.sync.dma_start(w1cb16[:], w1cb16_d.rearrange(
            "p (k m) -> p k m", k=KB16))
        w1db1_sb = const.tile([FEAT + 1, H], F32)
        nc.sync.dma_start(w1db1_sb[:], w1db1_d[:])
        distT1_sb = const.tile([FEAT + 1, NB], F32)
        nc.sync.dma_start(distT1_sb[:], distT1_d[:])
        sgnp = const.tile([128, MT], BF16)
        nc.sync.dma_start(sgnp[:], sgnp_d[:])

        # zdfb1[:, m, d] = S*(dist_emb[d] @ W1d + b1) for h_out tile m;
        # matmul with K = FEAT+1 (ones row carries b1).
        zdfb1 = const.tile([128, MT, NB], F32)
        ZAT = const.tile([128, MT, NLOC], F32)
        ZBTp = const.tile([128, MT, 2, W], F32)
        with tc.tile_pool(name="psum_pre", bufs=8, space="PSUM") as psum_pre:
            for m in range(MT):
                zp = psum_pre.tile([128, NB], F32, name=f"zp{m}", tag="zps",
                                   bufs=8)
                nc.tensor.matmul(
                    zp[:], w1db1_sb[:, m * 128:(m + 1) * 128], distT1_sb[:],
                    start=True, stop=True,
                )
                nc.vector.tensor_copy(zdfb1[:, m, :], zp[:])

            # ZAT[h_out, m, i] = S*(emb @ W1a)^T over current mentions;
            # ZBTp[h_out, m, j, c] = S*(emb @ W1b)^T over the window,
            # duplicated with a one-column shift for the paired-shift STT.
            for wi, wd in enumerate((w1a_d, w1b_d)):
                zps = [psum_pre.tile([128, W], F32, name=f"zps{wi}_{m}",
                                     tag="zps", bufs=8) for m in range(MT)]
                for k in range(KT):
                    wk = wab.tile([128, H], BF16, name=f"wk{wi}_{k}", tag="wk")
                    nc.sync.dma_start(wk[:], wd[k * 128:(k + 1) * 128, :])
                    for m in range(MT):
                        nc.tensor.matmul(
                            zps[m][:],
                            wk[:, m * 128:(m + 1) * 128],
                            embq[:, k, 0, :],
                            start=(k == 0), stop=(k == KT - 1),
                        )
                for m in range(MT):
                    if wi == 0:
                        nc.vector.tensor_copy(ZAT[:, m, :], zps[m][:, A:W])
                    else:
                        for j in range(2):
                            nc.vector.tensor_copy(
                                ZBTp[:, m, j, j:W], zps[m][:, 0:W - j])

        psum_main = ctx.enter_context(
            tc.tile_pool(name="psum_main", bufs=5, space="PSUM"))

        # Scrub the pending-zero state of every ps bank: the preamble's
        # start=True matmuls marked whole banks pending but cleared only the
        # bytes they wrote; a start=False accumulate onto a still-pending
        # address REPLACES (dropping the DVE preload). One full-width
        # start=True matmul per bank clears every address's pending bit.
        for d in range(5):
            scrub = psum_main.tile([128, 512], F32, name=f"scrub{d}",
                                   tag="ps", bufs=5)
            nc.tensor.matmul(
                scrub[:], w1cb16[:, 0, 0:128], w1cb16[:, 0, 0:512],
                start=True, stop=True,
            )

        groups = [list(range(g, min(g + GRP, NBLK)))
                  for g in range(0, NBLK, GRP)]
        for group in groups:
            xf8 = {}
            xb16 = {}
            for b in group:
                s0 = 2 * b + 1
                # X[h, (j, i)] = (2 emb[i]) * (2 emb[i-s0-j]), fp8 tiles
                # carry k-tiles [0, KF8), bf16 tiles the rest.
                if KF8:
                    xf8[b] = xpool.tile([128, KF8 // 2, 2, 2, 256], F8,
                                        name=f"Xf8_{b}", tag="xf8")
                if KB16:
                    xb16[b] = xpool.tile([128, KB16, 2, 256], BF16,
                                         name=f"Xb_{b}", tag="xb")
                for k in range(KT):
                    out = (xf8[b][:, k // 2, k % 2] if k < KF8
                           else xb16[b][:, k - KF8])
                    # Pool (gpsimd) is SBUF-only; give it the fp8 X tiles so
                    # DVE keeps capacity for the PSUM preloads.
                    eng = nc.gpsimd if k < KF8 else nc.vector
                    eng.tensor_mul(
                        out,
                        embq[:, k, 0:1, A:A + 256].broadcast_to(
                            [128, 2, 256]),
                        embq[:, k, 0:2, A - s0:A - s0 + 256],
                    )
            ps = {}
            for m in range(MT):
                for b in group:
                    s0 = 2 * b + 1
                    p = psum_main.tile([128, 512], F32, name=f"ps{b}_{m}",
                                       tag="ps", bufs=5)
                    ps[b] = p
                    # preload PSUM with S*(ZA + ZB + zdf + b1); matmuls
                    # accumulate the bilinear term on top. PSUM is only
                    # reachable from DVE/ACT; DVE does all preloads.
                    eng = nc.vector
                    if s0 >= 9:
                        eng.scalar_tensor_tensor(
                            p[:].rearrange("p (j i) -> p j i", j=2),
                            ZAT[:, m:m + 1, :].broadcast_to([128, 2, 256]),
                            zdfb1[:, m, 9:10],
                            ZBTp[:, m, 0:2, A - s0:A - s0 + 256],
                            ADD, ADD,
                        )
                    else:
                        for j in range(2):
                            s = s0 + j
                            eng.scalar_tensor_tensor(
                                p[:, j * 256:(j + 1) * 256],
                                ZAT[:, m, :],
                                zdfb1[:, m, min(s, 9):min(s, 9) + 1],
                                ZBTp[:, m, j, A - s0:A - s0 + 256],
                                ADD, ADD,
                            )
                for t in range(KF8 // 2):
                    for b in group:
                        nc.tensor.matmul(
                            ps[b][:],
                            w1cf8[:, t, :, m * 128:(m + 1) * 128],
                            xf8[b][:, t].rearrange("p s j i -> p s (j i)"),
                            start=False, stop=False, perf_mode=DR,
                            skip_group_check=True,
                        )
                for kk in range(KB16):
                    for b in group:
                        nc.tensor.matmul(
                            ps[b][:],
                            w1cb16[:, kk, m * 128:(m + 1) * 128],
                            xb16[b][:, kk].rearrange("p j i -> p (j i)"),
                            start=False, stop=(kk == KB16 - 1),
                            skip_group_check=True,
                        )
                for b in group:
                    if m == 0:
                        ht_b = htpool.tile([128, MT, 512], BF16,
                                           name=f"ht{b}", tag="ht")
                        ps[f"ht{b}"] = ht_b
                    nc.scalar.activation(ps[f"ht{b}"][:, m, :], ps[b][:],
                                         RELU)
            for b in group:
                sps = psum_main.tile([1, 512], F32, name=f"sps{b}",
                                     tag="sps", bufs=2)
                ht_b = ps[f"ht{b}"]
                for m in range(MT):
                    nc.tensor.matmul(
                        sps[:], sgnp[:, m:m + 1], ht_b[:, m, :],
                        start=(m == 0), stop=(m == MT - 1),
                    )
                srow = htpool.tile([1, 512], F32, name=f"srow{b}",
                                   tag="srow", bufs=2)
                nc.scalar.copy(srow[:], sps[:])
                nc.sync.dma_start(scores_d[b:b + 1, :], srow[:])

    nc.compile()
    if not nc.is_finalized():
        nc.finalize()
    return nc


def _host_prep(mention_embeddings, W1, b1, W2, dist_emb):
    emb = np.asarray(mention_embeddings, dtype=np.float32)
    W1 = np.asarray(W1, dtype=np.float32)
    b1 = np.asarray(b1, dtype=np.float32)
    W2 = np.asarray(W2, dtype=np.float32)
    dist_emb = np.asarray(dist_emb, dtype=np.float32)

    absw = np.abs(W2)
    sgn = np.sign(W2).astype(np.float32)
    W1s = W1 * absw[None, :]
    b1s = b1 * absw
    w1a, w1b, w1c = W1s[0:H], W1s[H:2 * H], W1s[2 * H:3 * H]

    # per-column power-of-two scale so fp8 w1c occupies (96, 192]
    colmax = np.maximum(np.abs(w1c).max(axis=0), 1e-30)
    S = (4.0 * 2.0 ** np.floor(np.log2(192.0 / colmax))).astype(np.float32)

    w1cS = w1c * (S[None, :] / 4.0)
    w1cf8 = np.zeros((128, (KF8 // 2) * 2 * H), ml_dtypes.float8_e4m3)
    v = w1cf8.reshape(128, KF8 // 2, 2, H)
    for t in range(KF8 // 2):
        for s in range(2):
            k = 2 * t + s
            v[:, t, s, :] = w1cS[k * 128:(k + 1) * 128, :].astype(
                ml_dtypes.float8_e4m3)
    w1cb16 = np.ascontiguousarray(
        w1cS[KF8 * 128:].reshape(KB16, 128, H).transpose(1, 0, 2).reshape(
            128, KB16 * H)).astype(ml_dtypes.bfloat16)

    w1a_in = (w1a * (S[None, :] / 2.0)).astype(ml_dtypes.bfloat16)
    w1b_in = (w1b * (S[None, :] / 2.0)).astype(ml_dtypes.bfloat16)
    w1db1 = np.concatenate([W1s[3 * H:], b1s[None, :]], axis=0) * S[None, :]
    w1db1 = np.ascontiguousarray(w1db1, dtype=np.float32)
    distT1 = np.ascontiguousarray(np.concatenate(
        [dist_emb.T, np.ones((1, NB), np.float32)], axis=0))
    sgnp = np.ascontiguousarray(
        (sgn / S).reshape(MT, 128).T).astype(ml_dtypes.bfloat16)

    # bf16 embedding window, x2 prescale, with a one-column-shift copy
    embTfull = np.zeros((N + A, H), dtype=np.float32)
    embTfull[A:] = emb * 2.0
    in_maps = []
    for r in range(NCORES):
        n0 = r * NLOC
        win = embTfull[n0:n0 + W]                       # [W, H]
        eq = np.zeros((128, KT, 2, W), ml_dtypes.bfloat16)
        wq = win.astype(ml_dtypes.bfloat16)             # [W, H]
        for k in range(KT):
            eq[:, k, 0, :] = wq[:, k * 128:(k + 1) * 128].T
            eq[:, k, 1, 1:] = wq[:W - 1, k * 128:(k + 1) * 128].T
        in_maps.append({
            "embq": np.ascontiguousarray(eq.reshape(128, KT * 2 * W)),
            "w1cf8": w1cf8, "w1cb16": w1cb16,
            "w1a": w1a_in, "w1b": w1b_in,
            "w1db1": w1db1, "distT1": distT1, "sgnp": sgnp,
        })
    return in_maps


def _assemble(grids, b2):
    """grids: list of 8 per-core [NBLK, 512] score arrays -> [N, A+1]."""
    b2v = np.float32(np.asarray(b2).reshape(-1)[0])
    # [50, 2048]: grid[s-1, i]
    grid = np.concatenate(
        [np.asarray(g, np.float32).reshape(A, NLOC) for g in grids], axis=1)
    out = np.zeros((N, A + 1), dtype=np.float32)
    big = grid[::-1].T + b2v          # big[i, c] = score(i, s=50-c) + b2
    out[A:, 1:] = big[A:]
    for i in range(1, A):
        ss = np.arange(1, i + 1)      # valid shifts for mention i < 50
        out[i, 1 + (i - ss)] = grid[ss - 1, i] + b2v
    return out


def kernel(mention_embeddings, mention_indices, max_antecedents, W1, b1, W2,
           b2, dist_emb):
    assert int(max_antecedents) == A
    in_maps = _host_prep(mention_embeddings, W1, b1, W2, dist_emb)
    nc = _build_nc()
    res = run_bass_kernel_spmd(nc, in_maps, list(range(NCORES)))
    grids = [res.results[r]["scores"] for r in range(NCORES)]
    return _assemble(grids, b2)
